# revision 41
# baseline (speedup 1.0000x reference)
"""Multi-head attention (B=8, S=1024, D=1024, H=16) on 8 TRN2 NeuronCores.

Sharding: pure data-parallel over batch — core b computes batch b entirely
locally (no collectives). All matmuls run in bf16 with fp32 PSUM accumulation
(fp8 V-projection was tried and rejected: e4m3 V quantization passes through
attention nearly undamped, ~2.5% output error vs the 2e-2 gate).

Per-core dataflow (host pre-transposes inputs/weights so no on-chip input
transposes are needed):
  Q_t[d,s], K_t[d,s] projected per d-tile (scale 1/sqrt(dk) folded into
  WQ/bq on the host; WQ/WK arrive as host-prearranged column slabs that
  stream through SBUF). V[s,d] is stored with a ones-column interleaved per
  head so the attention-value matmul also produces softmax row sums:
    S.T[k,q] = K_t_h.T @ Q_t_h    (K=64 matmul per 512-chunk)
    E.T = exp(S.T) * mask.T       (ACT exp from PSUM; mask mult on DVE,
                                   bf16 SBUF ops hit the fast 2x mode)
    psum[q, 0:65] = sum_k E.T_tile.T @ [V_h | 1]  -> out + rowsum
    attnout[q, d_h] = psum[:,0:64] * recip(psum[:,64])   (DVE)
  attnout transposed via PE -> WO projection -> + bias -> out[s,o] fp32.

Schedule (engine streams are static, so emission order IS the schedule).
The exp chain (ACT, 128 x ~1us) is the pacing resource; everything is
arranged so ACT starts early and never waits:
- DMA order: QK inputs first (sl_q, xq, slabs(1), xk -> first exp ~18us),
  then V tiles, masks; weight slabs prefetched one iteration ahead so the
  deep startup DMA queue never stalls the in-order PE queue.
- iteration 0: phase A = scores(head 0) interleaved with proj(1) steps,
  then the V projection batches (their tiles arrive behind the QK stream;
  2 groups in flight via psc halves + ppj), then phase B = scores(head 1)
  + V groups 6,7 — all of V completes before iteration 1's AVs.
- iterations t=1..7: per E-tile group i: score(2t,i) | proj(t+1) steps
  (q over groups 0-2, k over 3-5, one 2-bank ppj slot) | score(2t+1,i) |
  AV j-pairs of pair t-1 (i<4: head 2t-2, i>=4: head 2t-1) — every AV
  consumes exps emitted a full iteration earlier (epool holds 3 heads).
  Scores double-buffer through 2x2-bank psum (psc); AV psums + attnout
  transposes rotate through 2x1-bank slots (ps_av).
- transposes ride one iteration late (aot feeds only the epilogue WO) so
  they never gate an iteration boundary; the epilogue interleaves the last
  head's AVs with final transposes and per-chunk WO s-tiles (psum chunk ->
  bias add -> output DMA pipelined per 512 columns).
Cost-model (TimelineSim): ~225us/core vs 375us baseline; HW rel err 0.0033.
"""

import os
import sys
from contextlib import ExitStack

import numpy as np

if os.environ.get("JAX_PLATFORMS") == "cpu":
    # bass execution needs the neuron/axon jax backend
    del os.environ["JAX_PLATFORMS"]

for _p in ("/opt/trn_rl_repo",):
    if _p not in sys.path and os.path.isdir(_p):
        sys.path.insert(0, _p)

import ml_dtypes

import concourse.bass as bass
import concourse.mybir as mybir
import concourse.tile as tile
from concourse import bacc
from concourse.bass import ds, ts
from concourse.bass_utils import run_bass_kernel_spmd
from concourse.masks import make_identity

BF16 = mybir.dt.bfloat16
F32 = mybir.dt.float32
FP8 = mybir.dt.float8e4
NPBF = ml_dtypes.bfloat16
NPF8 = ml_dtypes.float8_e4m3

B, S, D, H, DK = 8, 1024, 1024, 16, 64
P = 128
NT = D // P  # 8 tiles along any 1024 dim
CH = 512  # matmul moving-dim chunk (one PSUM bank of fp32)
NCH = S // CH  # 2

MASK_ON_GPSIMD = False

LAST_RESULTS = None
_NC_CACHE = None


def build_nc():
    nc = bacc.Bacc("TRN2", target_bir_lowering=False, debug=False)

    xq = nc.dram_tensor("xq", [D, S], BF16, kind="ExternalInput")  # q[b].T
    xk = nc.dram_tensor("xk", [D, S], BF16, kind="ExternalInput")
    xv = nc.dram_tensor("xv", [D, S], BF16, kind="ExternalInput")
    # wq/wk: host-prearranged column slabs [t][p][i*128+f] = W.T[i*128+p, t*128+f]
    wq = nc.dram_tensor("wq", [NT, P, D], BF16, kind="ExternalInput")
    wk = nc.dram_tensor("wk", [NT, P, D], BF16, kind="ExternalInput")
    wv = nc.dram_tensor("wv", [D, D], BF16, kind="ExternalInput")  # WV_w.T
    wo = nc.dram_tensor("wo", [D, D], BF16, kind="ExternalInput")  # WO_w.T
    bq = nc.dram_tensor("bq", [P, NT], F32, kind="ExternalInput")  # WQ_b/8
    bk = nc.dram_tensor("bk", [P, NT], F32, kind="ExternalInput")
    bvb = nc.dram_tensor("bvb", [P, H * 65], BF16, kind="ExternalInput")
    bob = nc.dram_tensor("bob", [P, D], F32, kind="ExternalInput")
    mt = nc.dram_tensor("mt", [S, S], BF16, kind="ExternalInput")  # mask[b,0].T
    out = nc.dram_tensor("out", [S, D], F32, kind="ExternalOutput")

    with tile.TileContext(nc) as tc, ExitStack() as ctx:
        pers = ctx.enter_context(tc.tile_pool(name="pers", bufs=1))
        # xq+xk resident for the whole kernel
        xld = ctx.enter_context(tc.tile_pool(name="xld", bufs=16))
        xvp = ctx.enter_context(tc.tile_pool(name="xvp", bufs=8))
        # wv (early) then wo (late) share 8 slots
        wld = ctx.enter_context(tc.tile_pool(name="wld", bufs=8))
        wslab = ctx.enter_context(tc.tile_pool(name="wslab", bufs=4))
        # q/k projection outputs: only live for their head pair -> rotate
        qkp = ctx.enter_context(tc.tile_pool(name="qkp", bufs=3))
        # 3 heads of E tiles live at once (AV runs one head behind the exps)
        epool = ctx.enter_context(tc.tile_pool(name="epool", bufs=24))
        aop = ctx.enter_context(tc.tile_pool(name="aop", bufs=24))
        opool = ctx.enter_context(tc.tile_pool(name="opool", bufs=2))
        rpool = ctx.enter_context(tc.tile_pool(name="rpool", bufs=8))
        # psum (8 banks): scores double-buffer 2x[128,1024]f32 (4 banks),
        # projections 1x[128,1024]f32 (2 banks), AV + transposes 2x1 bank
        psc = ctx.enter_context(tc.tile_pool(name="psc", bufs=2, space="PSUM"))
        ppj = ctx.enter_context(tc.tile_pool(name="ppj", bufs=1, space="PSUM"))
        ps_av = ctx.enter_context(tc.tile_pool(name="ps_av", bufs=2, space="PSUM"))

        # ---- persistent tiles ----
        vv = [
            pers.tile([P, H * 65], BF16, name=f"vv{t}", tag=f"vv{t}")
            for t in range(NT)
        ]
        msk = [pers.tile([P, S], BF16, name=f"mk{t}", tag=f"mk{t}") for t in range(NT)]
        aot = [pers.tile([P, S], BF16, name=f"at{t}", tag=f"at{t}") for t in range(NT)]
        ident = pers.tile([P, P], BF16, name="ident", tag="ident")
        bq_sb = pers.tile([P, NT], F32, name="bq_sb", tag="bq_sb")
        bk_sb = pers.tile([P, NT], F32, name="bk_sb", tag="bk_sb")
        bv_sb = pers.tile([P, H * 65], BF16, name="bv_sb", tag="bv_sb")
        bo_sb = pers.tile([P, D], F32, name="bo_sb", tag="bo_sb")

        make_identity(nc, ident)

        def load_slab(wdram, ot):
            wsl = wslab.tile([P, D], BF16, name="wsl", tag="ws")
            nc.sync.dma_start(wsl[:], wdram[ot])
            return wsl

        # ---- input DMAs: V-path first — V-proj is the densest PE work per
        # DMA byte and fills the wire-paced startup; then the QK stream
        # (proj(0) gates the first exp), then masks. ----
        sl_q = load_slab(wq, 0)
        xqsb, xksb = [], []
        for i in range(NT):
            x_t = xld.tile([P, S], BF16, name=f"xq{i}", tag="x")
            nc.sync.dma_start(x_t[:], xq[ts(i, P), :])
            xqsb.append(x_t)
        nc.sync.dma_start(bq_sb[:], bq[:])
        nc.sync.dma_start(bk_sb[:], bk[:])
        sl1 = (load_slab(wq, 1), load_slab(wk, 1))
        sl_k = load_slab(wk, 0)
        for i in range(NT):
            x_t = xld.tile([P, S], BF16, name=f"xk{i}", tag="x")
            nc.sync.dma_start(x_t[:], xk[ts(i, P), :])
            xksb.append(x_t)
        wvsb = []
        xvsb = []
        for i in range(NT):
            w_t = wld.tile([P, D], BF16, name=f"wv{i}", tag="w")
            nc.sync.dma_start(w_t[:], wv[ts(i, P), :])
            wvsb.append(w_t)
            x_t = xvp.tile([P, S], BF16, name=f"xv{i}", tag="xv")
            nc.sync.dma_start(x_t[:], xv[ts(i, P), :])
            xvsb.append(x_t)
        nc.sync.dma_start(bv_sb[:], bvb[:])
        for i in range(NT):
            nc.sync.dma_start(msk[i][:], mt[ts(i, P), :])

        def project(wsl, bias, ot, xtiles, pname):
            """Full projection through two 1-bank psum chunks (pre-loop only)."""
            dst = qkp.tile([P, S], BF16, name=pname, tag=pname[0])
            for c in range(NCH):
                ps = psc.tile([P, CH], F32, name="ps_pj", tag="sc")
                for i in range(NT):
                    nc.tensor.matmul(
                        ps[:],
                        wsl[:, ts(i, P)],
                        xtiles[i][:, ts(c, CH)],
                        start=(i == 0),
                        stop=(i == NT - 1),
                    )
                nc.vector.tensor_scalar_add(
                    dst[:, ts(c, CH)], ps[:], bias[:, ds(ot, 1)]
                )
            return dst

        def score_tile(h, qt_t, kt_t, i, pool=None, tag="sc"):
            """scores -> exp -> mask for one [k-tile, q] slice of head h."""
            prow = (h % 2) * 64
            st_ps = (pool or psc).tile([P, S], F32, name="st", tag=tag)
            for c in range(NCH):
                nc.tensor.matmul(
                    st_ps[:, ts(c, CH)],
                    kt_t[ds(prow, 64), ts(i, P)],
                    qt_t[ds(prow, 64), ts(c, CH)],
                    start=True,
                    stop=True,
                )
            e = epool.tile([P, S], BF16, name=f"e{i}", tag="e")
            nc.scalar.activation(e[:], st_ps[:], mybir.ActivationFunctionType.Exp)
            # mask on DVE: bf16 SBUF-only tensor ops run in the fast 2x mode
            nc.vector.tensor_mul(e[:], e[:], msk[i][:])
            return e

        def av_j(h, eh, j, aopair):
            prow = (h % 2) * 64
            av = ps_av.tile([P, P], F32, name="av", tag="av")
            for i in range(NT):
                nc.tensor.matmul(
                    av[:, 0:65],
                    eh[i][:, ts(j, P)],
                    vv[i][:, ds(h * 65, 65)],
                    start=(i == 0),
                    stop=(i == NT - 1),
                )
            rc = rpool.tile([P, 1], F32, name="rc", tag="rc")
            nc.vector.reciprocal(rc[:], av[:, ds(64, 1)])
            # DVE (not GPSIMD): GPSIMD cannot read PSUM on real HW
            nc.vector.tensor_scalar_mul(aopair[j][:, ds(prow, 64)], av[:, 0:64], rc[:])

        def proj_step(ps, wsl, xtiles, i):
            for c in range(NCH):
                nc.tensor.matmul(
                    ps[:, ts(c, CH)],
                    wsl[:, ts(i, P)],
                    xtiles[i][:, ts(c, CH)],
                    start=(i == 0),
                    stop=(i == NT - 1),
                )

        def proj_evict(ps, bias, ot, pname):
            dst = qkp.tile([P, S], BF16, name=pname, tag=pname[0])
            for c in range(NCH):
                nc.vector.tensor_scalar_add(
                    dst[:, ts(c, CH)], ps[:, ts(c, CH)], bias[:, ds(ot, 1)]
                )
            return dst

        def transpose_pair(t, aopair):
            # all 8 [128,128]bf16 transposes fit ONE psum bank: 1 slot + 1 big
            # DVE copy instead of 8 of each — the next AV's psum slot frees
            # much sooner. j=0's start=True clears the bank's has_written bits
            # (stale from the slot's previous user); j>0 then overwrite their
            # untouched ranges.
            ptb = ps_av.tile([P, S], BF16, name="ptb", tag="av")
            for j in range(NT):
                nc.tensor.matmul(
                    ptb[:, ts(j, P)],
                    aopair[j][:],
                    ident[:],
                    is_transpose=True,
                    start=(j == 0),
                    stop=(j == NT - 1),
                    skip_group_check=True,
                )
            nc.vector.tensor_copy(aot[t][:], ptb[:])

        # ---- V projection (dense PE work during input DMA; AV depends on all
        # of V). V[s, d]: stationary = x.T [i,s]-tile, moving = W.T [i,o].
        # st-groups 0..5 run pre-loop, 3 per batch with interleaved i-loops
        # (the stream is paced by wv/xv DMA arrival, so 3-way interleave gives
        # PE 3x the work per arriving tile): 2 groups through psc as 1-bank
        # [128,512]f32 chunk pairs + 1 through ppj. Groups 6,7 run INSIDE
        # main-loop iteration 0 through the ps_av slots as the PE filler that
        # AVs provide in later iterations. ----
        def vstep_half(half, st_, i):
            for c in range(NCH):
                nc.tensor.matmul(
                    half[c][:],
                    xvsb[i][:, ts(st_, P)],
                    wvsb[i][:, ts(c, CH)],
                    start=(i == 0),
                    stop=(i == NT - 1),
                )

        def vevict_half(half, st_):
            # scatter 8 head-blocks of 64 into the 65-strided layout, + bias
            for c in range(NCH):
                g0c = c * 8
                nc.vector.tensor_add(
                    vv[st_][:, ds(g0c * 65, 8 * 65)].rearrange(
                        "p (g c) -> p g c", c=65
                    )[:, :, 0:64],
                    half[c].rearrange("p (g c) -> p g c", c=64),
                    bv_sb[:, ds(g0c * 65, 8 * 65)].rearrange(
                        "p (g c) -> p g c", c=65
                    )[:, :, 0:64],
                )

        for st_ in range(NT):
            nc.gpsimd.memset(
                vv[st_].rearrange("p (g c) -> p g c", c=65)[:, :, 64:65], 1.0
            )

        # ---- main loop: fine-grained interleave ----
        wo_partials = {}

        def wo_stile(j, wosb):
            # per-chunk psum (1 bank) + per-chunk eviction/DMA: pipelines the
            # output tail. s-tiles with an iteration-7 partial (i<=5 already
            # accumulated, bias folded) only add the i=6,7 contributions.
            osb = opool.tile([P, D], F32, name="osb", tag="osb")
            part = wo_partials.get(j)
            lo = 6 if part is not None else 0
            for c in range(NCH):
                ps = psc.tile([P, CH], F32, name="ps_wo", tag="sc")
                for i in range(lo, NT):
                    nc.tensor.matmul(
                        ps[:],
                        aot[i][:, ts(j, P)],
                        wosb[i][:, ts(c, CH)],
                        start=(i == lo),
                        stop=(i == NT - 1),
                    )
                if part is not None:
                    nc.vector.tensor_add(osb[:, ts(c, CH)], ps[:], part[:, ts(c, CH)])
                else:
                    nc.vector.tensor_add(
                        osb[:, ts(c, CH)], ps[:], bo_sb[:, ts(c, CH)]
                    )
                nc.sync.dma_start(out[ts(j, P), ts(c, CH)], osb[:, ts(c, CH)])

        qts = {0: project(sl_q, bq_sb, 0, xqsb, "qt")}
        kts = {0: project(sl_k, bk_sb, 0, xksb, "kt")}
        # slabs are DMA-queued one iteration ahead of use: the queue is deep
        # at startup and a just-in-time slab load would stall the PE queue
        slabs = {2: (load_slab(wq, 2), load_slab(wk, 2))}

        # Emission is round-robin per E-tile index i so PE always has ~2.1us
        # of score/projection/AV work per 2-exp ACT period (2.08us): per group
        #   score(2t, i) | proj steps | av(pair t-1) 2 j's | score(2t+1, i)
        # AVs consume exps emitted one full iteration earlier (epool holds 3-4
        # heads of E tiles); the i<4 groups retire head 2t-2, i>=4 head 2t-1.
        # The single proj psum slot carries q in groups 0..3, k in 4..7.
        wosb = []
        aopairs = {}
        ehs = {}
        # proj(t+1) runs at iteration t (one-ahead): q-steps over groups 0..2,
        # k-steps over 3..5 so kt evicts two groups before the next iteration
        # needs it. Iteration 0 instead runs the whole fp8 V projection in its
        # late groups (the V pair DMAs ride behind the QK input stream).
        q_sched = {0: (0, 1, 2), 1: (3, 4, 5), 2: (6, 7)}
        k_sched = {3: (0, 1), 4: (2, 3, 4), 5: (5, 6, 7)}

        # ---- iteration 0 (special): phase A = scores(head 0) + proj(1);
        # phase B = scores(head 1) + the whole fp8 V projection (V pair DMAs
        # arrive behind the QK input stream; groups alternate ps_av/ppj so two
        # are in flight) ----
        qt0, kt0 = qts.pop(0), kts.pop(0)
        aopairs[0] = [
            aop.tile([P, P], BF16, name=f"aop{j}", tag="aop") for j in range(NT)
        ]
        eh_a, eh_b = [], []
        qA = {0: (0, 1), 1: (2, 3), 2: (4, 5), 3: (6, 7)}
        kA = {4: (0, 1), 5: (2, 3), 6: (4, 5), 7: (6, 7)}
        pq = pk = None
        for i in range(NT):
            eh_a.append(score_tile(0, qt0, kt0, i))
            if i == 0:
                pq = ppj.tile([P, S], F32, name="ps_pj", tag="pj")
            for i_ in qA.get(i, ()):
                proj_step(pq, sl1[0], xqsb, i_)
            if i == 3:
                qts[1] = proj_evict(pq, bq_sb, 1, "qt")
                pk = ppj.tile([P, S], F32, name="ps_pj", tag="pj")
            for i_ in kA.get(i, ()):
                proj_step(pk, sl1[1], xksb, i_)
        kts[1] = proj_evict(pk, bk_sb, 1, "kt")
        for batch in ((0, 1), (2, 3), (4, 5)):
            halves = [
                psc.tile([P, CH], F32, name="ps_ph", tag="sc") for _ in range(NCH)
            ]
            pfull = ppj.tile([P, D], F32, name="ps_pv", tag="pj")
            for i in range(NT):
                vstep_half(halves, batch[0], i)
                for c in range(NCH):
                    nc.tensor.matmul(
                        pfull[:, ts(c, CH)],
                        xvsb[i][:, ts(batch[1], P)],
                        wvsb[i][:, ts(c, CH)],
                        start=(i == 0),
                        stop=(i == NT - 1),
                    )
            vevict_half(halves, batch[0])
            nc.vector.tensor_add(
                vv[batch[1]].rearrange("p (g c) -> p g c", c=65)[:, :, 0:64],
                pfull.rearrange("p (g c) -> p g c", c=64),
                bv_sb.rearrange("p (g c) -> p g c", c=65)[:, :, 0:64],
            )
        vh6 = [ps_av.tile([P, CH], F32, name="ps_ph", tag="av") for _ in range(NCH)]
        pf7 = ppj.tile([P, D], F32, name="ps_pv", tag="pj")
        for i in range(NT):
            eh_b.append(score_tile(1, qt0, kt0, i))
            vstep_half(vh6, 6, i)
            for c in range(NCH):
                nc.tensor.matmul(
                    pf7[:, ts(c, CH)],
                    xvsb[i][:, ts(7, P)],
                    wvsb[i][:, ts(c, CH)],
                    start=(i == 0),
                    stop=(i == NT - 1),
                )
        vevict_half(vh6, 6)
        nc.vector.tensor_add(
            vv[7].rearrange("p (g c) -> p g c", c=65)[:, :, 0:64],
            pf7.rearrange("p (g c) -> p g c", c=64),
            bv_sb.rearrange("p (g c) -> p g c", c=65)[:, :, 0:64],
        )
        ehs[0], ehs[1] = eh_a, eh_b

        for t in range(1, NT):
            qt_t, kt_t = qts.pop(t), kts.pop(t)
            aopairs[t] = [
                aop.tile([P, P], BF16, name=f"aop{j}", tag="aop") for j in range(NT)
            ]
            pt_ = t + 1
            do_proj = (t >= 1) and (pt_ <= NT - 1)
            if do_proj:
                sl_qt, sl_kt = slabs.pop(pt_)
                if pt_ + 1 < NT:
                    slabs[pt_ + 1] = (load_slab(wq, pt_ + 1), load_slab(wk, pt_ + 1))
            if t == 5:
                # prefetch WO weights (reuses the wv slots, long since free)
                nc.sync.dma_start(bo_sb[:], bob[:])
                for i in range(NT):
                    w_t = wld.tile([P, D], BF16, name=f"wo{i}", tag="w")
                    nc.sync.dma_start(w_t[:], wo[ts(i, P), :])
                    wosb.append(w_t)
            eh_a, eh_b = [], []
            eh_pa = ehs.pop(2 * t - 2, None)
            eh_pb = ehs.pop(2 * t - 1, None)
            pq = pk = None
            for i in range(NT):
                eh_a.append(score_tile(2 * t, qt_t, kt_t, i))
                if i == 1 and t > 1:
                    # transposes ride one iteration late (aot is only needed
                    # by the epilogue WO) so they never gate the iteration
                    # boundary's score emission
                    transpose_pair(t - 2, aopairs.pop(t - 2))
                if do_proj:
                    if i == 0:
                        pq = ppj.tile([P, S], F32, name="ps_pj", tag="pj")
                    for i_ in q_sched.get(i, ()):
                        proj_step(pq, sl_qt, xqsb, i_)
                    if i == 2:
                        qts[pt_] = proj_evict(pq, bq_sb, pt_, "qt")
                    if i == 3:
                        pk = ppj.tile([P, S], F32, name="ps_pj", tag="pj")
                    for i_ in k_sched.get(i, ()):
                        proj_step(pk, sl_kt, xksb, i_)
                    if i == 5:
                        kts[pt_] = proj_evict(pk, bk_sb, pt_, "kt")
                eh_b.append(score_tile(2 * t + 1, qt_t, kt_t, i))
                if eh_pa is not None:
                    if i < 4:
                        av_j(2 * t - 2, eh_pa, 2 * i, aopairs[t - 1])
                        av_j(2 * t - 2, eh_pa, 2 * i + 1, aopairs[t - 1])
                    else:
                        av_j(2 * t - 1, eh_pb, 2 * (i - 4), aopairs[t - 1])
                        av_j(2 * t - 1, eh_pb, 2 * (i - 4) + 1, aopairs[t - 1])
            ehs[2 * t] = eh_a
            ehs[2 * t + 1] = eh_b

        # ---- epilogue: last pair's AVs + pending transposes + WO ----
        eh_a = ehs.pop(2 * NT - 2)
        eh_b = ehs.pop(2 * NT - 1)
        aopair = aopairs.pop(NT - 1)
        transpose_pair(NT - 2, aopairs.pop(NT - 2))
        for j in range(NT):
            av_j(2 * NT - 2, eh_a, j, aopair)
        for j in range(NT + 2):
            if j < NT:
                av_j(2 * NT - 1, eh_b, j, aopair)
            if 1 <= j <= NT:
                pt = ps_av.tile([P, P], BF16, name="pt", tag="av")
                nc.tensor.transpose(pt[:], aopair[j - 1][:], ident[:])
                nc.vector.tensor_copy(aot[NT - 1][:, ts(j - 1, P)], pt[:])
            if j >= 2:
                wo_stile(j - 2, wosb)

    nc.compile()
    return nc


def prep_inputs(q, k, v, mask, WQ_w, WQ_b, WK_w, WK_b, WV_w, WV_b, WO_w, WO_b):
    """Build the 8 per-core input maps (host-side layout prep)."""
    f32 = np.float32

    def slabs(wt):  # [D,D] W.T -> [NT, P, D]: [t][p][i*128+f] = wt[i*128+p, t*128+f]
        return np.ascontiguousarray(
            wt.reshape(NT, P, NT, P).transpose(2, 1, 0, 3).reshape(NT, P, D)
        )

    wq_t = slabs((WQ_w.astype(f32) * 0.125).T).astype(NPBF)
    wk_t = slabs(WK_w.astype(f32).T).astype(NPBF)
    wv_t = np.ascontiguousarray(WV_w.astype(f32).T).astype(NPBF)
    wo_t = np.ascontiguousarray(WO_w.astype(f32).T).astype(NPBF)
    bq_l = np.ascontiguousarray((WQ_b.astype(f32) * 0.125).reshape(NT, P).T)
    bk_l = np.ascontiguousarray(WK_b.astype(f32).reshape(NT, P).T)
    bvb = np.zeros((P, H * 65), NPBF)
    bv_f = WV_b.astype(f32)
    for h in range(H):
        bvb[:, h * 65 : h * 65 + 64] = bv_f[h * 64 : (h + 1) * 64].astype(NPBF)[None, :]
    bob = np.ascontiguousarray(np.broadcast_to(WO_b.astype(f32), (P, D)))

    in_maps = []
    for b in range(B):
        in_maps.append(
            {
                "xq": np.ascontiguousarray(q[b].astype(f32).T).astype(NPBF),
                "xk": np.ascontiguousarray(k[b].astype(f32).T).astype(NPBF),
                "xv": np.ascontiguousarray(v[b].astype(f32).T).astype(NPBF),
                "wq": wq_t,
                "wk": wk_t,
                "wv": wv_t,
                "wo": wo_t,
                "bq": bq_l,
                "bk": bk_l,
                "bvb": bvb,
                "bob": bob,
                "mt": np.ascontiguousarray(mask[b, 0].T.astype(f32)).astype(NPBF),
            }
        )
    return in_maps


def _ensure_neuron_backend():
    # if jax was already initialized cpu-only (e.g. JAX_PLATFORMS=cpu was set
    # before this module was imported), re-discover the neuron/axon backend
    import jax

    try:
        if all(d.platform == "cpu" for d in jax.devices()):
            jax.clear_backends()
    except Exception:
        pass


def kernel(q, k, v, mask, WQ_w, WQ_b, WK_w, WK_b, WV_w, WV_b, WO_w, WO_b):
    global _NC_CACHE, LAST_RESULTS
    _ensure_neuron_backend()
    if _NC_CACHE is None:
        _NC_CACHE = build_nc()
    nc = _NC_CACHE
    in_maps = prep_inputs(
        q, k, v, mask, WQ_w, WQ_b, WK_w, WK_b, WV_w, WV_b, WO_w, WO_b
    )
    res = run_bass_kernel_spmd(nc, in_maps, core_ids=list(range(B)))
    LAST_RESULTS = res
    out = np.stack([res.results[b]["out"] for b in range(B)], axis=0).astype(np.float32)
    if not np.isfinite(out).all():
        # very first execution on a freshly attached core has been seen to
        # return garbage once; one retry clears it
        res = run_bass_kernel_spmd(nc, in_maps, core_ids=list(range(B)))
        LAST_RESULTS = res
        out = np.stack([res.results[b]["out"] for b in range(B)], axis=0).astype(
            np.float32
        )
    return out



# revision 42
# speedup vs baseline: 1.0913x; 1.0913x over previous
"""Multi-head attention (B=8, S=1024, D=1024, H=16) on 8 TRN2 NeuronCores.

Sharding: pure data-parallel over batch — core b computes batch b entirely
locally (no collectives). All matmuls run in bf16 with fp32 PSUM accumulation
(fp8 V-projection was tried and rejected: e4m3 V quantization passes through
attention nearly undamped, ~2.5% output error vs the 2e-2 gate).

Per-core dataflow (host pre-transposes inputs/weights so no on-chip input
transposes are needed):
  Q_t[d,s], K_t[d,s] projected per d-tile (scale 1/sqrt(dk) folded into
  WQ/bq on the host; WQ/WK arrive as host-prearranged column slabs that
  stream through SBUF). V[s,d] is stored with a ones-column interleaved per
  head so the attention-value matmul also produces softmax row sums:
    S.T[k,q] = K_t_h.T @ Q_t_h    (K=64 matmul per 512-chunk)
    E.T = exp(S.T) * mask.T       (ACT exp from PSUM; mask mult on DVE,
                                   bf16 SBUF ops hit the fast 2x mode)
    psum[q, 0:65] = sum_k E.T_tile.T @ [V_h | 1]  -> out + rowsum
    attnout[q, d_h] = psum[:,0:64] * recip(psum[:,64])   (DVE)
  attnout transposed via PE -> WO projection -> + bias -> out[s,o] fp32.

Schedule (engine streams are static, so emission order IS the schedule).
The exp chain (ACT, 128 x ~1us) is the pacing resource; everything is
arranged so ACT starts early and never waits:
- DMA order: QK inputs first (sl_q, xq, slabs(1), xk -> first exp ~18us),
  then V tiles, masks; weight slabs prefetched one iteration ahead so the
  deep startup DMA queue never stalls the in-order PE queue.
- iteration 0: phase A = scores(head 0) interleaved with proj(1) steps,
  then the V projection batches (their tiles arrive behind the QK stream;
  2 groups in flight via psc halves + ppj), then phase B = scores(head 1)
  + V groups 6,7 — all of V completes before iteration 1's AVs.
- iterations t=1..7: per E-tile group i: score(2t,i) | proj(t+1) steps
  (q over groups 0-2, k over 3-5, one 2-bank ppj slot) | score(2t+1,i) |
  AV j-pairs of pair t-1 (i<4: head 2t-2, i>=4: head 2t-1) — every AV
  consumes exps emitted a full iteration earlier (epool holds 3 heads).
  Scores double-buffer through 2x2-bank psum (psc); AV psums + attnout
  transposes rotate through 2x1-bank slots (ps_av).
- transposes ride one iteration late (aot feeds only the epilogue WO) so
  they never gate an iteration boundary; the epilogue interleaves the last
  head's AVs with final transposes and per-chunk WO s-tiles (psum chunk ->
  bias add -> output DMA pipelined per 512 columns).
Cost-model (TimelineSim): ~225us/core vs 375us baseline; HW rel err 0.0033.
"""

import os
import sys
from contextlib import ExitStack

import numpy as np

if os.environ.get("JAX_PLATFORMS") == "cpu":
    # bass execution needs the neuron/axon jax backend
    del os.environ["JAX_PLATFORMS"]

for _p in ("/opt/trn_rl_repo",):
    if _p not in sys.path and os.path.isdir(_p):
        sys.path.insert(0, _p)

import ml_dtypes

import concourse.bass as bass
import concourse.mybir as mybir
import concourse.tile as tile
from concourse import bacc
from concourse.bass import ds, ts
from concourse.bass_utils import run_bass_kernel_spmd
from concourse.masks import make_identity

BF16 = mybir.dt.bfloat16
F32 = mybir.dt.float32
FP8 = mybir.dt.float8e4
NPBF = ml_dtypes.bfloat16
NPF8 = ml_dtypes.float8_e4m3

B, S, D, H, DK = 8, 1024, 1024, 16, 64
P = 128
NT = D // P  # 8 tiles along any 1024 dim
CH = 512  # matmul moving-dim chunk (one PSUM bank of fp32)
NCH = S // CH  # 2

MASK_ON_GPSIMD = False

LAST_RESULTS = None
_NC_CACHE = None


def build_nc():
    nc = bacc.Bacc("TRN2", target_bir_lowering=False, debug=False)

    xq = nc.dram_tensor("xq", [D, S], BF16, kind="ExternalInput")  # q[b].T
    xk = nc.dram_tensor("xk", [D, S], BF16, kind="ExternalInput")
    xv = nc.dram_tensor("xv", [D, S], BF16, kind="ExternalInput")
    # wq/wk: host-prearranged column slabs [t][p][i*128+f] = W.T[i*128+p, t*128+f]
    wq = nc.dram_tensor("wq", [NT, P, D], BF16, kind="ExternalInput")
    wk = nc.dram_tensor("wk", [NT, P, D], BF16, kind="ExternalInput")
    wv = nc.dram_tensor("wv", [D, D], BF16, kind="ExternalInput")  # WV_w.T
    wo = nc.dram_tensor("wo", [D, D], BF16, kind="ExternalInput")  # WO_w.T
    bq = nc.dram_tensor("bq", [P, NT], F32, kind="ExternalInput")  # WQ_b/8
    bk = nc.dram_tensor("bk", [P, NT], F32, kind="ExternalInput")
    bvb = nc.dram_tensor("bvb", [P, H * 65], BF16, kind="ExternalInput")
    bob = nc.dram_tensor("bob", [P, D], F32, kind="ExternalInput")
    mt = nc.dram_tensor("mt", [S, S], BF16, kind="ExternalInput")  # mask[b,0].T
    out = nc.dram_tensor("out", [S, D], F32, kind="ExternalOutput")

    with tile.TileContext(nc) as tc, ExitStack() as ctx:
        pers = ctx.enter_context(tc.tile_pool(name="pers", bufs=1))
        # xq+xk resident for the whole kernel
        xld = ctx.enter_context(tc.tile_pool(name="xld", bufs=16))
        xvp = ctx.enter_context(tc.tile_pool(name="xvp", bufs=8))
        # wv (early) then wo (late) share 8 slots
        wld = ctx.enter_context(tc.tile_pool(name="wld", bufs=8))
        wslab = ctx.enter_context(tc.tile_pool(name="wslab", bufs=4))
        # q/k projection outputs: only live for their head pair -> rotate
        qkp = ctx.enter_context(tc.tile_pool(name="qkp", bufs=3))
        # 3 heads of E tiles live at once (AV runs one head behind the exps)
        epool = ctx.enter_context(tc.tile_pool(name="epool", bufs=24))
        aop = ctx.enter_context(tc.tile_pool(name="aop", bufs=24))
        opool = ctx.enter_context(tc.tile_pool(name="opool", bufs=2))
        rpool = ctx.enter_context(tc.tile_pool(name="rpool", bufs=8))
        # psum (8 banks): scores double-buffer 2x[128,1024]f32 (4 banks),
        # projections 1x[128,1024]f32 (2 banks), AV + transposes 2x1 bank
        psc = ctx.enter_context(tc.tile_pool(name="psc", bufs=2, space="PSUM"))
        ppj = ctx.enter_context(tc.tile_pool(name="ppj", bufs=1, space="PSUM"))
        ps_av = ctx.enter_context(tc.tile_pool(name="ps_av", bufs=2, space="PSUM"))

        # ---- persistent tiles ----
        vv = [
            pers.tile([P, H * 65], BF16, name=f"vv{t}", tag=f"vv{t}")
            for t in range(NT)
        ]
        msk = [pers.tile([P, S], BF16, name=f"mk{t}", tag=f"mk{t}") for t in range(NT)]
        aot = [pers.tile([P, S], BF16, name=f"at{t}", tag=f"at{t}") for t in range(NT)]
        ident = pers.tile([P, P], BF16, name="ident", tag="ident")
        bq_sb = pers.tile([P, NT], F32, name="bq_sb", tag="bq_sb")
        bk_sb = pers.tile([P, NT], F32, name="bk_sb", tag="bk_sb")
        bv_sb = pers.tile([P, H * 65], BF16, name="bv_sb", tag="bv_sb")
        bo_sb = pers.tile([P, D], F32, name="bo_sb", tag="bo_sb")

        make_identity(nc, ident)

        def load_slab(wdram, ot):
            wsl = wslab.tile([P, D], BF16, name="wsl", tag="ws")
            nc.sync.dma_start(wsl[:], wdram[ot])
            return wsl

        # ---- input DMAs: V-path first — V-proj is the densest PE work per
        # DMA byte and fills the wire-paced startup; then the QK stream
        # (proj(0) gates the first exp), then masks. ----
        sl_q = load_slab(wq, 0)
        xqsb, xksb = [], []
        for i in range(NT):
            x_t = xld.tile([P, S], BF16, name=f"xq{i}", tag="x")
            nc.sync.dma_start(x_t[:], xq[ts(i, P), :])
            xqsb.append(x_t)
        nc.sync.dma_start(bq_sb[:], bq[:])
        nc.sync.dma_start(bk_sb[:], bk[:])
        sl1 = (load_slab(wq, 1), load_slab(wk, 1))
        sl_k = load_slab(wk, 0)
        for i in range(NT):
            x_t = xld.tile([P, S], BF16, name=f"xk{i}", tag="x")
            nc.sync.dma_start(x_t[:], xk[ts(i, P), :])
            xksb.append(x_t)
        wvsb = []
        xvsb = []
        for i in range(NT):
            w_t = wld.tile([P, D], BF16, name=f"wv{i}", tag="w")
            nc.sync.dma_start(w_t[:], wv[ts(i, P), :])
            wvsb.append(w_t)
            x_t = xvp.tile([P, S], BF16, name=f"xv{i}", tag="xv")
            nc.sync.dma_start(x_t[:], xv[ts(i, P), :])
            xvsb.append(x_t)
        nc.sync.dma_start(bv_sb[:], bvb[:])
        for i in range(NT):
            nc.sync.dma_start(msk[i][:], mt[ts(i, P), :])

        def project(wsl, bias, ot, xtiles, pname):
            """Full projection through two 1-bank psum chunks (pre-loop only)."""
            dst = qkp.tile([P, S], BF16, name=pname, tag=pname[0])
            for c in range(NCH):
                ps = psc.tile([P, CH], F32, name="ps_pj", tag="sc")
                for i in range(NT):
                    nc.tensor.matmul(
                        ps[:],
                        wsl[:, ts(i, P)],
                        xtiles[i][:, ts(c, CH)],
                        start=(i == 0),
                        stop=(i == NT - 1),
                    )
                nc.vector.tensor_scalar_add(
                    dst[:, ts(c, CH)], ps[:], bias[:, ds(ot, 1)]
                )
            return dst

        def score_tile(h, qt_t, kt_t, i, pool=None, tag="sc"):
            """scores -> exp -> mask for one [k-tile, q] slice of head h."""
            prow = (h % 2) * 64
            st_ps = (pool or psc).tile([P, S], F32, name="st", tag=tag)
            for c in range(NCH):
                nc.tensor.matmul(
                    st_ps[:, ts(c, CH)],
                    kt_t[ds(prow, 64), ts(i, P)],
                    qt_t[ds(prow, 64), ts(c, CH)],
                    start=True,
                    stop=True,
                )
            e = epool.tile([P, S], BF16, name=f"e{i}", tag="e")
            nc.scalar.activation(e[:], st_ps[:], mybir.ActivationFunctionType.Exp)
            # mask on DVE: bf16 SBUF-only tensor ops run in the fast 2x mode
            nc.vector.tensor_mul(e[:], e[:], msk[i][:])
            return e

        def av_j(h, eh, j, aopair):
            prow = (h % 2) * 64
            av = ps_av.tile([P, P], F32, name="av", tag="av")
            for i in range(NT):
                nc.tensor.matmul(
                    av[:, 0:65],
                    eh[i][:, ts(j, P)],
                    vv[i][:, ds(h * 65, 65)],
                    start=(i == 0),
                    stop=(i == NT - 1),
                )
            rc = rpool.tile([P, 1], F32, name="rc", tag="rc")
            nc.vector.reciprocal(rc[:], av[:, ds(64, 1)])
            # DVE (not GPSIMD): GPSIMD cannot read PSUM on real HW
            nc.vector.tensor_scalar_mul(aopair[j][:, ds(prow, 64)], av[:, 0:64], rc[:])

        def proj_step(ps, wsl, xtiles, i):
            for c in range(NCH):
                nc.tensor.matmul(
                    ps[:, ts(c, CH)],
                    wsl[:, ts(i, P)],
                    xtiles[i][:, ts(c, CH)],
                    start=(i == 0),
                    stop=(i == NT - 1),
                )

        def proj_evict(ps, bias, ot, pname):
            dst = qkp.tile([P, S], BF16, name=pname, tag=pname[0])
            for c in range(NCH):
                nc.vector.tensor_scalar_add(
                    dst[:, ts(c, CH)], ps[:, ts(c, CH)], bias[:, ds(ot, 1)]
                )
            return dst

        def transpose_pair(t, aopair):
            # all 8 [128,128]bf16 transposes fit ONE psum bank: 1 slot + 1 big
            # DVE copy instead of 8 of each — the next AV's psum slot frees
            # much sooner. j=0's start=True clears the bank's has_written bits
            # (stale from the slot's previous user); j>0 then overwrite their
            # untouched ranges.
            ptb = ps_av.tile([P, S], BF16, name="ptb", tag="av")
            for j in range(NT):
                nc.tensor.matmul(
                    ptb[:, ts(j, P)],
                    aopair[j][:],
                    ident[:],
                    is_transpose=True,
                    start=(j == 0),
                    stop=(j == NT - 1),
                    skip_group_check=True,
                )
            nc.vector.tensor_copy(aot[t][:], ptb[:])

        # ---- V projection (dense PE work during input DMA; AV depends on all
        # of V). V[s, d]: stationary = x.T [i,s]-tile, moving = W.T [i,o].
        # st-groups 0..5 run pre-loop, 3 per batch with interleaved i-loops
        # (the stream is paced by wv/xv DMA arrival, so 3-way interleave gives
        # PE 3x the work per arriving tile): 2 groups through psc as 1-bank
        # [128,512]f32 chunk pairs + 1 through ppj. Groups 6,7 run INSIDE
        # main-loop iteration 0 through the ps_av slots as the PE filler that
        # AVs provide in later iterations. ----
        def vstep_half(half, st_, i):
            for c in range(NCH):
                nc.tensor.matmul(
                    half[c][:],
                    xvsb[i][:, ts(st_, P)],
                    wvsb[i][:, ts(c, CH)],
                    start=(i == 0),
                    stop=(i == NT - 1),
                )

        def vevict_half(half, st_):
            # scatter 8 head-blocks of 64 into the 65-strided layout, + bias
            for c in range(NCH):
                g0c = c * 8
                nc.vector.tensor_add(
                    vv[st_][:, ds(g0c * 65, 8 * 65)].rearrange(
                        "p (g c) -> p g c", c=65
                    )[:, :, 0:64],
                    half[c].rearrange("p (g c) -> p g c", c=64),
                    bv_sb[:, ds(g0c * 65, 8 * 65)].rearrange(
                        "p (g c) -> p g c", c=65
                    )[:, :, 0:64],
                )

        for st_ in range(NT):
            nc.gpsimd.memset(
                vv[st_].rearrange("p (g c) -> p g c", c=65)[:, :, 64:65], 1.0
            )

        # ---- main loop: fine-grained interleave ----
        wo_partials = {}

        def wo_stile(j, wosb):
            # per-chunk psum (1 bank) + per-chunk eviction/DMA: pipelines the
            # output tail. s-tiles with an iteration-7 partial (i<=5 already
            # accumulated, bias folded) only add the i=6,7 contributions.
            osb = opool.tile([P, D], F32, name="osb", tag="osb")
            part = wo_partials.get(j)
            lo = 6 if part is not None else 0
            for c in range(NCH):
                ps = psc.tile([P, CH], F32, name="ps_wo", tag="sc")
                for i in range(lo, NT):
                    nc.tensor.matmul(
                        ps[:],
                        aot[i][:, ts(j, P)],
                        wosb[i][:, ts(c, CH)],
                        start=(i == lo),
                        stop=(i == NT - 1),
                    )
                if part is not None:
                    nc.vector.tensor_add(osb[:, ts(c, CH)], ps[:], part[:, ts(c, CH)])
                else:
                    nc.vector.tensor_add(
                        osb[:, ts(c, CH)], ps[:], bo_sb[:, ts(c, CH)]
                    )
                nc.sync.dma_start(out[ts(j, P), ts(c, CH)], osb[:, ts(c, CH)])

        qts = {0: project(sl_q, bq_sb, 0, xqsb, "qt")}
        kts = {0: project(sl_k, bk_sb, 0, xksb, "kt")}
        # slabs are DMA-queued one iteration ahead of use: the queue is deep
        # at startup and a just-in-time slab load would stall the PE queue
        slabs = {2: (load_slab(wq, 2), load_slab(wk, 2))}

        # Emission is round-robin per E-tile index i so PE always has ~2.1us
        # of score/projection/AV work per 2-exp ACT period (2.08us): per group
        #   score(2t, i) | proj steps | av(pair t-1) 2 j's | score(2t+1, i)
        # AVs consume exps emitted one full iteration earlier (epool holds 3-4
        # heads of E tiles); the i<4 groups retire head 2t-2, i>=4 head 2t-1.
        # The single proj psum slot carries q in groups 0..3, k in 4..7.
        wosb = []
        aopairs = {}
        ehs = {}
        # proj(t+1) runs at iteration t (one-ahead): q-steps over groups 0..2,
        # k-steps over 3..5 so kt evicts two groups before the next iteration
        # needs it. Iteration 0 instead runs the whole fp8 V projection in its
        # late groups (the V pair DMAs ride behind the QK input stream).
        q_sched = {0: (0, 1, 2), 1: (3, 4, 5), 2: (6, 7)}
        k_sched = {3: (0, 1), 4: (2, 3, 4), 5: (5, 6, 7)}

        # ---- iteration 0 (special): phase A = scores(head 0) + proj(1);
        # phase B = scores(head 1) + the whole fp8 V projection (V pair DMAs
        # arrive behind the QK input stream; groups alternate ps_av/ppj so two
        # are in flight) ----
        qt0, kt0 = qts.pop(0), kts.pop(0)
        aopairs[0] = [
            aop.tile([P, P], BF16, name=f"aop{j}", tag="aop") for j in range(NT)
        ]
        eh_a, eh_b = [], []
        qA = {0: (0, 1), 1: (2, 3), 2: (4, 5), 3: (6, 7)}
        kA = {4: (0, 1), 5: (2, 3), 6: (4, 5), 7: (6, 7)}
        pq = pk = None
        for i in range(NT):
            eh_a.append(score_tile(0, qt0, kt0, i))
            if i == 0:
                pq = ppj.tile([P, S], F32, name="ps_pj", tag="pj")
            for i_ in qA.get(i, ()):
                proj_step(pq, sl1[0], xqsb, i_)
            if i == 3:
                qts[1] = proj_evict(pq, bq_sb, 1, "qt")
                pk = ppj.tile([P, S], F32, name="ps_pj", tag="pj")
            for i_ in kA.get(i, ()):
                proj_step(pk, sl1[1], xksb, i_)
        kts[1] = proj_evict(pk, bk_sb, 1, "kt")
        for batch in ((0, 1), (2, 3), (4, 5)):
            halves = [
                psc.tile([P, CH], F32, name="ps_ph", tag="sc") for _ in range(NCH)
            ]
            pfull = ppj.tile([P, D], F32, name="ps_pv", tag="pj")
            for i in range(NT):
                vstep_half(halves, batch[0], i)
                for c in range(NCH):
                    nc.tensor.matmul(
                        pfull[:, ts(c, CH)],
                        xvsb[i][:, ts(batch[1], P)],
                        wvsb[i][:, ts(c, CH)],
                        start=(i == 0),
                        stop=(i == NT - 1),
                    )
            vevict_half(halves, batch[0])
            nc.vector.tensor_add(
                vv[batch[1]].rearrange("p (g c) -> p g c", c=65)[:, :, 0:64],
                pfull.rearrange("p (g c) -> p g c", c=64),
                bv_sb.rearrange("p (g c) -> p g c", c=65)[:, :, 0:64],
            )
        vh6 = [ps_av.tile([P, CH], F32, name="ps_ph", tag="av") for _ in range(NCH)]
        pf7 = ppj.tile([P, D], F32, name="ps_pv", tag="pj")
        for i in range(NT):
            eh_b.append(score_tile(1, qt0, kt0, i))
            vstep_half(vh6, 6, i)
            for c in range(NCH):
                nc.tensor.matmul(
                    pf7[:, ts(c, CH)],
                    xvsb[i][:, ts(7, P)],
                    wvsb[i][:, ts(c, CH)],
                    start=(i == 0),
                    stop=(i == NT - 1),
                )
        vevict_half(vh6, 6)
        nc.vector.tensor_add(
            vv[7].rearrange("p (g c) -> p g c", c=65)[:, :, 0:64],
            pf7.rearrange("p (g c) -> p g c", c=64),
            bv_sb.rearrange("p (g c) -> p g c", c=65)[:, :, 0:64],
        )
        ehs[0], ehs[1] = eh_a, eh_b

        for t in range(1, NT):
            qt_t, kt_t = qts.pop(t), kts.pop(t)
            aopairs[t] = [
                aop.tile([P, P], BF16, name=f"aop{j}", tag="aop") for j in range(NT)
            ]
            pt_ = t + 1
            do_proj = (t >= 1) and (pt_ <= NT - 1)
            if do_proj:
                sl_qt, sl_kt = slabs.pop(pt_)
                if pt_ + 1 < NT:
                    slabs[pt_ + 1] = (load_slab(wq, pt_ + 1), load_slab(wk, pt_ + 1))
            if t == 5:
                # prefetch WO weights (reuses the wv slots, long since free)
                nc.sync.dma_start(bo_sb[:], bob[:])
                for i in range(NT):
                    w_t = wld.tile([P, D], BF16, name=f"wo{i}", tag="w")
                    nc.sync.dma_start(w_t[:], wo[ts(i, P), :])
                    wosb.append(w_t)
            eh_a, eh_b = [], []
            eh_pa = ehs.pop(2 * t - 2, None)
            eh_pb = ehs.pop(2 * t - 1, None)
            pq = pk = None
            if t < NT - 1:
                for i in range(NT):
                    eh_a.append(score_tile(2 * t, qt_t, kt_t, i))
                    if i == 1 and t > 1:
                        # transposes ride one iteration late (aot is only
                        # needed by the epilogue WO) so they never gate the
                        # iteration boundary's score emission
                        transpose_pair(t - 2, aopairs.pop(t - 2))
                    if do_proj:
                        if i == 0:
                            pq = ppj.tile([P, S], F32, name="ps_pj", tag="pj")
                        for i_ in q_sched.get(i, ()):
                            proj_step(pq, sl_qt, xqsb, i_)
                        if i == 2:
                            qts[pt_] = proj_evict(pq, bq_sb, pt_, "qt")
                        if i == 3:
                            pk = ppj.tile([P, S], F32, name="ps_pj", tag="pj")
                        for i_ in k_sched.get(i, ()):
                            proj_step(pk, sl_kt, xksb, i_)
                        if i == 5:
                            kts[pt_] = proj_evict(pk, bk_sb, pt_, "kt")
                    eh_b.append(score_tile(2 * t + 1, qt_t, kt_t, i))
                    if eh_pa is not None:
                        if i < 4:
                            av_j(2 * t - 2, eh_pa, 2 * i, aopairs[t - 1])
                            av_j(2 * t - 2, eh_pa, 2 * i + 1, aopairs[t - 1])
                        else:
                            av_j(2 * t - 1, eh_pb, 2 * (i - 4), aopairs[t - 1])
                            av_j(2 * t - 1, eh_pb, 2 * (i - 4) + 1, aopairs[t - 1])
            else:
                # iteration 7 (no proj work): front-load head-14's scores into
                # groups 0-3 so ACT finishes exps(14) mid-iteration, letting
                # its AVs (otherwise epilogue work) fill the late groups
                for i in range(4):
                    eh_a.append(score_tile(2 * t, qt_t, kt_t, 2 * i))
                    eh_a.append(score_tile(2 * t, qt_t, kt_t, 2 * i + 1))
                    if i == 1:
                        transpose_pair(t - 2, aopairs.pop(t - 2))
                    av_j(2 * t - 2, eh_pa, 2 * i, aopairs[t - 1])
                    av_j(2 * t - 2, eh_pa, 2 * i + 1, aopairs[t - 1])
                av14 = {4: (), 5: (0, 1, 2), 6: (3, 4, 5), 7: (6, 7)}
                for i in range(4, NT):
                    eh_b.append(score_tile(2 * t + 1, qt_t, kt_t, 2 * (i - 4)))
                    eh_b.append(score_tile(2 * t + 1, qt_t, kt_t, 2 * (i - 4) + 1))
                    av_j(2 * t - 1, eh_pb, 2 * (i - 4), aopairs[t - 1])
                    av_j(2 * t - 1, eh_pb, 2 * (i - 4) + 1, aopairs[t - 1])
                    for jj in av14[i]:
                        av_j(2 * t, eh_a, jj, aopairs[t])
            ehs[2 * t] = eh_a
            ehs[2 * t + 1] = eh_b

        # ---- epilogue: final AVs + pending transposes + WO ----
        eh_a = ehs.pop(2 * NT - 2)  # consumed in iteration 7 already
        eh_b = ehs.pop(2 * NT - 1)
        aopair = aopairs.pop(NT - 1)
        transpose_pair(NT - 2, aopairs.pop(NT - 2))
        for j in range(NT + 2):
            if j < NT:
                av_j(2 * NT - 1, eh_b, j, aopair)
            if 1 <= j <= NT:
                pt = ps_av.tile([P, P], BF16, name="pt", tag="av")
                nc.tensor.transpose(pt[:], aopair[j - 1][:], ident[:])
                nc.vector.tensor_copy(aot[NT - 1][:, ts(j - 1, P)], pt[:])
            if j >= 2:
                wo_stile(j - 2, wosb)

    nc.compile()
    return nc


def prep_inputs(q, k, v, mask, WQ_w, WQ_b, WK_w, WK_b, WV_w, WV_b, WO_w, WO_b):
    """Build the 8 per-core input maps (host-side layout prep)."""
    f32 = np.float32

    def slabs(wt):  # [D,D] W.T -> [NT, P, D]: [t][p][i*128+f] = wt[i*128+p, t*128+f]
        return np.ascontiguousarray(
            wt.reshape(NT, P, NT, P).transpose(2, 1, 0, 3).reshape(NT, P, D)
        )

    wq_t = slabs((WQ_w.astype(f32) * 0.125).T).astype(NPBF)
    wk_t = slabs(WK_w.astype(f32).T).astype(NPBF)
    wv_t = np.ascontiguousarray(WV_w.astype(f32).T).astype(NPBF)
    wo_t = np.ascontiguousarray(WO_w.astype(f32).T).astype(NPBF)
    bq_l = np.ascontiguousarray((WQ_b.astype(f32) * 0.125).reshape(NT, P).T)
    bk_l = np.ascontiguousarray(WK_b.astype(f32).reshape(NT, P).T)
    bvb = np.zeros((P, H * 65), NPBF)
    bv_f = WV_b.astype(f32)
    for h in range(H):
        bvb[:, h * 65 : h * 65 + 64] = bv_f[h * 64 : (h + 1) * 64].astype(NPBF)[None, :]
    bob = np.ascontiguousarray(np.broadcast_to(WO_b.astype(f32), (P, D)))

    in_maps = []
    for b in range(B):
        in_maps.append(
            {
                "xq": np.ascontiguousarray(q[b].astype(f32).T).astype(NPBF),
                "xk": np.ascontiguousarray(k[b].astype(f32).T).astype(NPBF),
                "xv": np.ascontiguousarray(v[b].astype(f32).T).astype(NPBF),
                "wq": wq_t,
                "wk": wk_t,
                "wv": wv_t,
                "wo": wo_t,
                "bq": bq_l,
                "bk": bk_l,
                "bvb": bvb,
                "bob": bob,
                "mt": np.ascontiguousarray(mask[b, 0].T.astype(f32)).astype(NPBF),
            }
        )
    return in_maps


def _ensure_neuron_backend():
    # if jax was already initialized cpu-only (e.g. JAX_PLATFORMS=cpu was set
    # before this module was imported), re-discover the neuron/axon backend
    import jax

    try:
        if all(d.platform == "cpu" for d in jax.devices()):
            jax.clear_backends()
    except Exception:
        pass


def kernel(q, k, v, mask, WQ_w, WQ_b, WK_w, WK_b, WV_w, WV_b, WO_w, WO_b):
    global _NC_CACHE, LAST_RESULTS
    _ensure_neuron_backend()
    if _NC_CACHE is None:
        _NC_CACHE = build_nc()
    nc = _NC_CACHE
    in_maps = prep_inputs(
        q, k, v, mask, WQ_w, WQ_b, WK_w, WK_b, WV_w, WV_b, WO_w, WO_b
    )
    res = run_bass_kernel_spmd(nc, in_maps, core_ids=list(range(B)))
    LAST_RESULTS = res
    out = np.stack([res.results[b]["out"] for b in range(B)], axis=0).astype(np.float32)
    if not np.isfinite(out).all():
        # very first execution on a freshly attached core has been seen to
        # return garbage once; one retry clears it
        res = run_bass_kernel_spmd(nc, in_maps, core_ids=list(range(B)))
        LAST_RESULTS = res
        out = np.stack([res.results[b]["out"] for b in range(B)], axis=0).astype(
            np.float32
        )
    return out



# revision 51
# speedup vs baseline: 1.2301x; 1.1271x over previous
"""Multi-head attention (B=8, S=1024, D=1024, H=16) on 8 TRN2 NeuronCores.

Sharding: pure data-parallel over batch — core b computes batch b entirely
locally (no collectives). All matmuls run in bf16 with fp32 PSUM accumulation
(fp8 V-projection was tried and rejected: e4m3 V quantization passes through
attention nearly undamped, ~2.5% output error vs the 2e-2 gate).

Per-core dataflow (host pre-transposes inputs/weights so no on-chip input
transposes are needed):
  Q_t[d,s], K_t[d,s] projected per d-tile (scale 1/sqrt(dk) folded into
  WQ/bq on the host; WQ/WK arrive as host-prearranged column slabs that
  stream through SBUF). V[s,d] is stored with a ones-column interleaved per
  head so the attention-value matmul also produces softmax row sums:
    S.T[k,q] = K_t_h.T @ Q_t_h    (K=64 matmul per 512-chunk)
    E.T = exp(S.T) * mask.T       (ACT exp from PSUM; mask mult on DVE,
                                   bf16 SBUF ops hit the fast 2x mode)
    psum[q, 0:65] = sum_k E.T_tile.T @ [V_h | 1]  -> out + rowsum
    attnout[q, d_h] = psum[:,0:64] * recip(psum[:,64])   (DVE)
  attnout transposed via PE -> WO projection -> + bias -> out[s,o] fp32.

Schedule (engine streams are static, so emission order IS the schedule).
The exp chain (ACT, 128 x ~1us) is the pacing resource; everything is
arranged so ACT starts early and never waits:
- DMA order: QK inputs first (sl_q, xq, slabs(1), xk -> first exp ~18us),
  then V tiles, masks; weight slabs prefetched one iteration ahead so the
  deep startup DMA queue never stalls the in-order PE queue.
- iteration 0: phase A = scores(head 0) interleaved with proj(1) steps,
  then the V projection batches (their tiles arrive behind the QK stream;
  2 groups in flight via psc halves + ppj), then phase B = scores(head 1)
  + V groups 6,7 — all of V completes before iteration 1's AVs.
- iterations t=1..7: per E-tile group i: score(2t,i) | proj(t+1) steps
  (q over groups 0-2, k over 3-5, one 2-bank ppj slot) | score(2t+1,i) |
  AV j-pairs of pair t-1 (i<4: head 2t-2, i>=4: head 2t-1) — every AV
  consumes exps emitted a full iteration earlier (epool holds 3 heads).
  Scores double-buffer through 2x2-bank psum (psc); AV psums + attnout
  transposes rotate through 2x1-bank slots (ps_av).
- transposes ride one iteration late (aot feeds only the epilogue WO) so
  they never gate an iteration boundary. Iteration 7 (projection-less)
  front-loads head-14's scores into groups 0-3 so ACT finishes their exps
  mid-iteration and head-14's AVs fill the late groups; the epilogue then
  interleaves only head-15's AVs with the final transposes and per-chunk WO
  s-tiles (psum chunk -> bias add -> output DMA pipelined per 512 columns).
Cost-model (TimelineSim): ~223us/core vs 375us baseline; HW rel err 0.0033.
"""

import os
import sys
from contextlib import ExitStack

import numpy as np

if os.environ.get("JAX_PLATFORMS") == "cpu":
    # bass execution needs the neuron/axon jax backend
    del os.environ["JAX_PLATFORMS"]

for _p in ("/opt/trn_rl_repo",):
    if _p not in sys.path and os.path.isdir(_p):
        sys.path.insert(0, _p)

import ml_dtypes

import concourse.bass as bass
import concourse.mybir as mybir
import concourse.tile as tile
from concourse import bacc
from concourse.bass import ds, ts
from concourse.bass_utils import run_bass_kernel_spmd
from concourse.masks import make_identity

BF16 = mybir.dt.bfloat16
F32 = mybir.dt.float32
FP8 = mybir.dt.float8e4
NPBF = ml_dtypes.bfloat16
NPF8 = ml_dtypes.float8_e4m3

B, S, D, H, DK = 8, 1024, 1024, 16, 64
P = 128
NT = D // P  # 8 tiles along any 1024 dim
CH = 512  # matmul moving-dim chunk (one PSUM bank of fp32)
NCH = S // CH  # 2

MASK_ON_GPSIMD = False

LAST_RESULTS = None
_NC_CACHE = None


def build_nc():
    nc = bacc.Bacc("TRN2", target_bir_lowering=False, debug=False)

    xq = nc.dram_tensor("xq", [D, S], BF16, kind="ExternalInput")  # q[b].T
    xk = nc.dram_tensor("xk", [D, S], BF16, kind="ExternalInput")
    xv = nc.dram_tensor("xv", [D, S], BF16, kind="ExternalInput")
    # wq/wk: host-prearranged column slabs [t][p][i*128+f] = W.T[i*128+p, t*128+f]
    wq = nc.dram_tensor("wq", [NT, P, D], BF16, kind="ExternalInput")
    wk = nc.dram_tensor("wk", [NT, P, D], BF16, kind="ExternalInput")
    wv = nc.dram_tensor("wv", [D, D], BF16, kind="ExternalInput")  # WV_w.T
    wo = nc.dram_tensor("wo", [D, D], BF16, kind="ExternalInput")  # WO_w.T
    bq = nc.dram_tensor("bq", [P, NT], F32, kind="ExternalInput")  # WQ_b/8
    bk = nc.dram_tensor("bk", [P, NT], F32, kind="ExternalInput")
    bvb = nc.dram_tensor("bvb", [P, H * 65], BF16, kind="ExternalInput")
    bob = nc.dram_tensor("bob", [P, D], F32, kind="ExternalInput")
    mt = nc.dram_tensor("mt", [S, S], BF16, kind="ExternalInput")  # mask[b,0].T
    # bf16 output: halves the on-chip out-DMA and the device->host fetch;
    # the host upcasts to f32 (costs ~+0.04% rel err vs the 2e-2 gate)
    out = nc.dram_tensor("out", [S, D], BF16, kind="ExternalOutput")

    with tile.TileContext(nc) as tc, ExitStack() as ctx:
        pers = ctx.enter_context(tc.tile_pool(name="pers", bufs=1))
        # xq+xk resident for the whole kernel
        xld = ctx.enter_context(tc.tile_pool(name="xld", bufs=16))
        xvp = ctx.enter_context(tc.tile_pool(name="xvp", bufs=8))
        # wv (early) then wo (late) share 8 slots
        wld = ctx.enter_context(tc.tile_pool(name="wld", bufs=8))
        wslab = ctx.enter_context(tc.tile_pool(name="wslab", bufs=4))
        # q/k projection outputs: only live for their head pair -> rotate
        qkp = ctx.enter_context(tc.tile_pool(name="qkp", bufs=3))
        # 3 heads of E tiles live at once (AV runs one head behind the exps)
        epool = ctx.enter_context(tc.tile_pool(name="epool", bufs=24))
        aop = ctx.enter_context(tc.tile_pool(name="aop", bufs=24))
        opool = ctx.enter_context(tc.tile_pool(name="opool", bufs=2))
        rpool = ctx.enter_context(tc.tile_pool(name="rpool", bufs=8))
        # psum (8 banks): scores double-buffer 2x[128,1024]f32 (4 banks),
        # projections 1x[128,1024]f32 (2 banks), AV + transposes 2x1 bank
        psc = ctx.enter_context(tc.tile_pool(name="psc", bufs=2, space="PSUM"))
        ppj = ctx.enter_context(tc.tile_pool(name="ppj", bufs=1, space="PSUM"))
        ps_av = ctx.enter_context(tc.tile_pool(name="ps_av", bufs=2, space="PSUM"))

        # ---- persistent tiles ----
        vv = [
            pers.tile([P, H * 65], BF16, name=f"vv{t}", tag=f"vv{t}")
            for t in range(NT)
        ]
        msk = [pers.tile([P, S], BF16, name=f"mk{t}", tag=f"mk{t}") for t in range(NT)]
        aot = [pers.tile([P, S], BF16, name=f"at{t}", tag=f"at{t}") for t in range(NT)]
        ident = pers.tile([P, P], BF16, name="ident", tag="ident")
        bq_sb = pers.tile([P, NT], F32, name="bq_sb", tag="bq_sb")
        bk_sb = pers.tile([P, NT], F32, name="bk_sb", tag="bk_sb")
        bv_sb = pers.tile([P, H * 65], BF16, name="bv_sb", tag="bv_sb")
        bo_sb = pers.tile([P, D], F32, name="bo_sb", tag="bo_sb")

        make_identity(nc, ident)

        def load_slab(wdram, ot):
            wsl = wslab.tile([P, D], BF16, name="wsl", tag="ws")
            nc.sync.dma_start(wsl[:], wdram[ot])
            return wsl

        # ---- input DMAs: V-path first — V-proj is the densest PE work per
        # DMA byte and fills the wire-paced startup; then the QK stream
        # (proj(0) gates the first exp), then masks. ----
        sl_q = load_slab(wq, 0)
        xqsb, xksb = [], []
        for i in range(NT):
            x_t = xld.tile([P, S], BF16, name=f"xq{i}", tag="x")
            nc.sync.dma_start(x_t[:], xq[ts(i, P), :])
            xqsb.append(x_t)
        nc.sync.dma_start(bq_sb[:], bq[:])
        nc.sync.dma_start(bk_sb[:], bk[:])
        sl1 = (load_slab(wq, 1), load_slab(wk, 1))
        sl_k = load_slab(wk, 0)
        for i in range(NT):
            x_t = xld.tile([P, S], BF16, name=f"xk{i}", tag="x")
            nc.sync.dma_start(x_t[:], xk[ts(i, P), :])
            xksb.append(x_t)
        wvsb = []
        xvsb = []
        for i in range(NT):
            w_t = wld.tile([P, D], BF16, name=f"wv{i}", tag="w")
            nc.sync.dma_start(w_t[:], wv[ts(i, P), :])
            wvsb.append(w_t)
            x_t = xvp.tile([P, S], BF16, name=f"xv{i}", tag="xv")
            nc.sync.dma_start(x_t[:], xv[ts(i, P), :])
            xvsb.append(x_t)
        nc.sync.dma_start(bv_sb[:], bvb[:])
        for i in range(NT):
            nc.sync.dma_start(msk[i][:], mt[ts(i, P), :])

        def project(wsl, bias, ot, xtiles, pname):
            """Full projection through two 1-bank psum chunks (pre-loop only)."""
            dst = qkp.tile([P, S], BF16, name=pname, tag=pname[0])
            for c in range(NCH):
                ps = psc.tile([P, CH], F32, name="ps_pj", tag="sc")
                for i in range(NT):
                    nc.tensor.matmul(
                        ps[:],
                        wsl[:, ts(i, P)],
                        xtiles[i][:, ts(c, CH)],
                        start=(i == 0),
                        stop=(i == NT - 1),
                    )
                nc.vector.tensor_scalar_add(
                    dst[:, ts(c, CH)], ps[:], bias[:, ds(ot, 1)]
                )
            return dst

        def score_tile(h, qt_t, kt_t, i, pool=None, tag="sc"):
            """scores -> exp -> mask for one [k-tile, q] slice of head h."""
            prow = (h % 2) * 64
            st_ps = (pool or psc).tile([P, S], F32, name="st", tag=tag)
            for c in range(NCH):
                nc.tensor.matmul(
                    st_ps[:, ts(c, CH)],
                    kt_t[ds(prow, 64), ts(i, P)],
                    qt_t[ds(prow, 64), ts(c, CH)],
                    start=True,
                    stop=True,
                )
            e = epool.tile([P, S], BF16, name=f"e{i}", tag="e")
            nc.scalar.activation(e[:], st_ps[:], mybir.ActivationFunctionType.Exp)
            # mask on DVE: bf16 SBUF-only tensor ops run in the fast 2x mode
            nc.vector.tensor_mul(e[:], e[:], msk[i][:])
            return e

        def av_j(h, eh, j, aopair):
            prow = (h % 2) * 64
            av = ps_av.tile([P, P], F32, name="av", tag="av")
            for i in range(NT):
                nc.tensor.matmul(
                    av[:, 0:65],
                    eh[i][:, ts(j, P)],
                    vv[i][:, ds(h * 65, 65)],
                    start=(i == 0),
                    stop=(i == NT - 1),
                )
            rc = rpool.tile([P, 1], F32, name="rc", tag="rc")
            nc.vector.reciprocal(rc[:], av[:, ds(64, 1)])
            # DVE (not GPSIMD): GPSIMD cannot read PSUM on real HW
            nc.vector.tensor_scalar_mul(aopair[j][:, ds(prow, 64)], av[:, 0:64], rc[:])

        def proj_step(ps, wsl, xtiles, i):
            for c in range(NCH):
                nc.tensor.matmul(
                    ps[:, ts(c, CH)],
                    wsl[:, ts(i, P)],
                    xtiles[i][:, ts(c, CH)],
                    start=(i == 0),
                    stop=(i == NT - 1),
                )

        def proj_evict(ps, bias, ot, pname):
            dst = qkp.tile([P, S], BF16, name=pname, tag=pname[0])
            for c in range(NCH):
                nc.vector.tensor_scalar_add(
                    dst[:, ts(c, CH)], ps[:, ts(c, CH)], bias[:, ds(ot, 1)]
                )
            return dst

        def transpose_pair(t, aopair):
            # all 8 [128,128]bf16 transposes fit ONE psum bank: 1 slot + 1 big
            # DVE copy instead of 8 of each — the next AV's psum slot frees
            # much sooner. j=0's start=True clears the bank's has_written bits
            # (stale from the slot's previous user); j>0 then overwrite their
            # untouched ranges.
            ptb = ps_av.tile([P, S], BF16, name="ptb", tag="av")
            for j in range(NT):
                nc.tensor.matmul(
                    ptb[:, ts(j, P)],
                    aopair[j][:],
                    ident[:],
                    is_transpose=True,
                    start=(j == 0),
                    stop=(j == NT - 1),
                    skip_group_check=True,
                )
            nc.vector.tensor_copy(aot[t][:], ptb[:])

        # ---- V projection (dense PE work during input DMA; AV depends on all
        # of V). V[s, d]: stationary = x.T [i,s]-tile, moving = W.T [i,o].
        # st-groups 0..5 run pre-loop, 3 per batch with interleaved i-loops
        # (the stream is paced by wv/xv DMA arrival, so 3-way interleave gives
        # PE 3x the work per arriving tile): 2 groups through psc as 1-bank
        # [128,512]f32 chunk pairs + 1 through ppj. Groups 6,7 run INSIDE
        # main-loop iteration 0 through the ps_av slots as the PE filler that
        # AVs provide in later iterations. ----
        def vstep_half(half, st_, i):
            for c in range(NCH):
                nc.tensor.matmul(
                    half[c][:],
                    xvsb[i][:, ts(st_, P)],
                    wvsb[i][:, ts(c, CH)],
                    start=(i == 0),
                    stop=(i == NT - 1),
                )

        def vevict_half(half, st_):
            # scatter 8 head-blocks of 64 into the 65-strided layout, + bias
            for c in range(NCH):
                g0c = c * 8
                nc.vector.tensor_add(
                    vv[st_][:, ds(g0c * 65, 8 * 65)].rearrange(
                        "p (g c) -> p g c", c=65
                    )[:, :, 0:64],
                    half[c].rearrange("p (g c) -> p g c", c=64),
                    bv_sb[:, ds(g0c * 65, 8 * 65)].rearrange(
                        "p (g c) -> p g c", c=65
                    )[:, :, 0:64],
                )

        for st_ in range(NT):
            nc.gpsimd.memset(
                vv[st_].rearrange("p (g c) -> p g c", c=65)[:, :, 64:65], 1.0
            )

        # ---- main loop: fine-grained interleave ----
        wo_partials = {}

        def wo_stile(j, wosb):
            # per-chunk psum (1 bank) + per-chunk eviction/DMA: pipelines the
            # output tail. s-tiles with an iteration-7 partial (i<=5 already
            # accumulated, bias folded) only add the i=6,7 contributions.
            osb = opool.tile([P, D], BF16, name="osb", tag="osb")
            part = wo_partials.get(j)
            lo = 6 if part is not None else 0
            for c in range(NCH):
                ps = psc.tile([P, CH], F32, name="ps_wo", tag="sc")
                for i in range(lo, NT):
                    nc.tensor.matmul(
                        ps[:],
                        aot[i][:, ts(j, P)],
                        wosb[i][:, ts(c, CH)],
                        start=(i == lo),
                        stop=(i == NT - 1),
                    )
                if part is not None:
                    nc.vector.tensor_add(osb[:, ts(c, CH)], ps[:], part[:, ts(c, CH)])
                else:
                    nc.vector.tensor_add(
                        osb[:, ts(c, CH)], ps[:], bo_sb[:, ts(c, CH)]
                    )
                nc.sync.dma_start(out[ts(j, P), ts(c, CH)], osb[:, ts(c, CH)])

        qts = {0: project(sl_q, bq_sb, 0, xqsb, "qt")}
        kts = {0: project(sl_k, bk_sb, 0, xksb, "kt")}
        # slabs are DMA-queued one iteration ahead of use: the queue is deep
        # at startup and a just-in-time slab load would stall the PE queue
        slabs = {2: (load_slab(wq, 2), load_slab(wk, 2))}

        # Emission is round-robin per E-tile index i so PE always has ~2.1us
        # of score/projection/AV work per 2-exp ACT period (2.08us): per group
        #   score(2t, i) | proj steps | av(pair t-1) 2 j's | score(2t+1, i)
        # AVs consume exps emitted one full iteration earlier (epool holds 3-4
        # heads of E tiles); the i<4 groups retire head 2t-2, i>=4 head 2t-1.
        # The single proj psum slot carries q in groups 0..3, k in 4..7.
        wosb = []
        aopairs = {}
        ehs = {}
        # proj(t+1) runs at iteration t (one-ahead): q-steps over groups 0..2,
        # k-steps over 3..5 so kt evicts two groups before the next iteration
        # needs it. Iteration 0 instead runs the whole fp8 V projection in its
        # late groups (the V pair DMAs ride behind the QK input stream).
        q_sched = {0: (0, 1, 2), 1: (3, 4, 5), 2: (6, 7)}
        k_sched = {3: (0, 1), 4: (2, 3, 4), 5: (5, 6, 7)}

        # ---- iteration 0 (special): phase A = scores(head 0) + proj(1);
        # phase B = scores(head 1) + the whole fp8 V projection (V pair DMAs
        # arrive behind the QK input stream; groups alternate ps_av/ppj so two
        # are in flight) ----
        qt0, kt0 = qts.pop(0), kts.pop(0)
        aopairs[0] = [
            aop.tile([P, P], BF16, name=f"aop{j}", tag="aop") for j in range(NT)
        ]
        eh_a, eh_b = [], []
        qA = {0: (0, 1), 1: (2, 3), 2: (4, 5), 3: (6, 7)}
        kA = {4: (0, 1), 5: (2, 3), 6: (4, 5), 7: (6, 7)}
        pq = pk = None
        for i in range(NT):
            eh_a.append(score_tile(0, qt0, kt0, i))
            if i == 0:
                pq = ppj.tile([P, S], F32, name="ps_pj", tag="pj")
            for i_ in qA.get(i, ()):
                proj_step(pq, sl1[0], xqsb, i_)
            if i == 3:
                qts[1] = proj_evict(pq, bq_sb, 1, "qt")
                pk = ppj.tile([P, S], F32, name="ps_pj", tag="pj")
            for i_ in kA.get(i, ()):
                proj_step(pk, sl1[1], xksb, i_)
        kts[1] = proj_evict(pk, bk_sb, 1, "kt")
        for batch in ((0, 1), (2, 3), (4, 5)):
            halves = [
                psc.tile([P, CH], F32, name="ps_ph", tag="sc") for _ in range(NCH)
            ]
            pfull = ppj.tile([P, D], F32, name="ps_pv", tag="pj")
            for i in range(NT):
                vstep_half(halves, batch[0], i)
                for c in range(NCH):
                    nc.tensor.matmul(
                        pfull[:, ts(c, CH)],
                        xvsb[i][:, ts(batch[1], P)],
                        wvsb[i][:, ts(c, CH)],
                        start=(i == 0),
                        stop=(i == NT - 1),
                    )
            vevict_half(halves, batch[0])
            nc.vector.tensor_add(
                vv[batch[1]].rearrange("p (g c) -> p g c", c=65)[:, :, 0:64],
                pfull.rearrange("p (g c) -> p g c", c=64),
                bv_sb.rearrange("p (g c) -> p g c", c=65)[:, :, 0:64],
            )
        vh6 = [ps_av.tile([P, CH], F32, name="ps_ph", tag="av") for _ in range(NCH)]
        pf7 = ppj.tile([P, D], F32, name="ps_pv", tag="pj")
        for i in range(NT):
            eh_b.append(score_tile(1, qt0, kt0, i))
            vstep_half(vh6, 6, i)
            for c in range(NCH):
                nc.tensor.matmul(
                    pf7[:, ts(c, CH)],
                    xvsb[i][:, ts(7, P)],
                    wvsb[i][:, ts(c, CH)],
                    start=(i == 0),
                    stop=(i == NT - 1),
                )
        vevict_half(vh6, 6)
        nc.vector.tensor_add(
            vv[7].rearrange("p (g c) -> p g c", c=65)[:, :, 0:64],
            pf7.rearrange("p (g c) -> p g c", c=64),
            bv_sb.rearrange("p (g c) -> p g c", c=65)[:, :, 0:64],
        )
        ehs[0], ehs[1] = eh_a, eh_b

        for t in range(1, NT):
            qt_t, kt_t = qts.pop(t), kts.pop(t)
            aopairs[t] = [
                aop.tile([P, P], BF16, name=f"aop{j}", tag="aop") for j in range(NT)
            ]
            pt_ = t + 1
            do_proj = (t >= 1) and (pt_ <= NT - 1)
            if do_proj:
                sl_qt, sl_kt = slabs.pop(pt_)
                if pt_ + 1 < NT:
                    slabs[pt_ + 1] = (load_slab(wq, pt_ + 1), load_slab(wk, pt_ + 1))
            if t == 5:
                # prefetch WO weights (reuses the wv slots, long since free)
                nc.sync.dma_start(bo_sb[:], bob[:])
                for i in range(NT):
                    w_t = wld.tile([P, D], BF16, name=f"wo{i}", tag="w")
                    nc.sync.dma_start(w_t[:], wo[ts(i, P), :])
                    wosb.append(w_t)
            eh_a, eh_b = [], []
            eh_pa = ehs.pop(2 * t - 2, None)
            eh_pb = ehs.pop(2 * t - 1, None)
            pq = pk = None
            if t < NT - 1:
                for i in range(NT):
                    eh_a.append(score_tile(2 * t, qt_t, kt_t, i))
                    if i == 1 and t > 1:
                        # transposes ride one iteration late (aot is only
                        # needed by the epilogue WO) so they never gate the
                        # iteration boundary's score emission
                        transpose_pair(t - 2, aopairs.pop(t - 2))
                    if do_proj:
                        if i == 0:
                            pq = ppj.tile([P, S], F32, name="ps_pj", tag="pj")
                        for i_ in q_sched.get(i, ()):
                            proj_step(pq, sl_qt, xqsb, i_)
                        if i == 2:
                            qts[pt_] = proj_evict(pq, bq_sb, pt_, "qt")
                        if i == 3:
                            pk = ppj.tile([P, S], F32, name="ps_pj", tag="pj")
                        for i_ in k_sched.get(i, ()):
                            proj_step(pk, sl_kt, xksb, i_)
                        if i == 5:
                            kts[pt_] = proj_evict(pk, bk_sb, pt_, "kt")
                    eh_b.append(score_tile(2 * t + 1, qt_t, kt_t, i))
                    if eh_pa is not None:
                        if i < 4:
                            av_j(2 * t - 2, eh_pa, 2 * i, aopairs[t - 1])
                            av_j(2 * t - 2, eh_pa, 2 * i + 1, aopairs[t - 1])
                        else:
                            av_j(2 * t - 1, eh_pb, 2 * (i - 4), aopairs[t - 1])
                            av_j(2 * t - 1, eh_pb, 2 * (i - 4) + 1, aopairs[t - 1])
            else:
                # iteration 7 (no proj work): front-load head-14's scores into
                # groups 0-3 so ACT finishes exps(14) mid-iteration, letting
                # its AVs (otherwise epilogue work) fill the late groups
                for i in range(4):
                    eh_a.append(score_tile(2 * t, qt_t, kt_t, 2 * i))
                    eh_a.append(score_tile(2 * t, qt_t, kt_t, 2 * i + 1))
                    if i == 1:
                        transpose_pair(t - 2, aopairs.pop(t - 2))
                    av_j(2 * t - 2, eh_pa, 2 * i, aopairs[t - 1])
                    av_j(2 * t - 2, eh_pa, 2 * i + 1, aopairs[t - 1])
                av14 = {4: (), 5: (0, 1, 2), 6: (3, 4, 5), 7: (6, 7)}
                for i in range(4, NT):
                    eh_b.append(score_tile(2 * t + 1, qt_t, kt_t, 2 * (i - 4)))
                    eh_b.append(score_tile(2 * t + 1, qt_t, kt_t, 2 * (i - 4) + 1))
                    av_j(2 * t - 1, eh_pb, 2 * (i - 4), aopairs[t - 1])
                    av_j(2 * t - 1, eh_pb, 2 * (i - 4) + 1, aopairs[t - 1])
                    for jj in av14[i]:
                        av_j(2 * t, eh_a, jj, aopairs[t])
            ehs[2 * t] = eh_a
            ehs[2 * t + 1] = eh_b

        # ---- epilogue: final AVs + pending transposes + WO ----
        eh_a = ehs.pop(2 * NT - 2)  # consumed in iteration 7 already
        eh_b = ehs.pop(2 * NT - 1)
        aopair = aopairs.pop(NT - 1)
        transpose_pair(NT - 2, aopairs.pop(NT - 2))
        for j in range(NT + 2):
            if j < NT:
                av_j(2 * NT - 1, eh_b, j, aopair)
            if 1 <= j <= NT:
                pt = ps_av.tile([P, P], BF16, name="pt", tag="av")
                nc.tensor.transpose(pt[:], aopair[j - 1][:], ident[:])
                nc.vector.tensor_copy(aot[NT - 1][:, ts(j - 1, P)], pt[:])
            if j >= 2:
                wo_stile(j - 2, wosb)

    nc.compile()
    return nc


def prep_inputs(q, k, v, mask, WQ_w, WQ_b, WK_w, WK_b, WV_w, WV_b, WO_w, WO_b):
    """Build the 8 per-core input maps (host-side layout prep)."""
    f32 = np.float32

    def slabs(wt):  # [D,D] W.T -> [NT, P, D]: [t][p][i*128+f] = wt[i*128+p, t*128+f]
        return np.ascontiguousarray(
            wt.reshape(NT, P, NT, P).transpose(2, 1, 0, 3).reshape(NT, P, D)
        )

    wq_t = slabs((WQ_w.astype(f32) * 0.125).T).astype(NPBF)
    wk_t = slabs(WK_w.astype(f32).T).astype(NPBF)
    wv_t = np.ascontiguousarray(WV_w.astype(f32).T).astype(NPBF)
    wo_t = np.ascontiguousarray(WO_w.astype(f32).T).astype(NPBF)
    bq_l = np.ascontiguousarray((WQ_b.astype(f32) * 0.125).reshape(NT, P).T)
    bk_l = np.ascontiguousarray(WK_b.astype(f32).reshape(NT, P).T)
    bvb = np.zeros((P, H * 65), NPBF)
    bv_f = WV_b.astype(f32)
    for h in range(H):
        bvb[:, h * 65 : h * 65 + 64] = bv_f[h * 64 : (h + 1) * 64].astype(NPBF)[None, :]
    bob = np.ascontiguousarray(np.broadcast_to(WO_b.astype(f32), (P, D)))

    in_maps = []
    for b in range(B):
        in_maps.append(
            {
                "xq": np.ascontiguousarray(q[b].astype(f32).T).astype(NPBF),
                "xk": np.ascontiguousarray(k[b].astype(f32).T).astype(NPBF),
                "xv": np.ascontiguousarray(v[b].astype(f32).T).astype(NPBF),
                "wq": wq_t,
                "wk": wk_t,
                "wv": wv_t,
                "wo": wo_t,
                "bq": bq_l,
                "bk": bk_l,
                "bvb": bvb,
                "bob": bob,
                "mt": np.ascontiguousarray(mask[b, 0].T.astype(f32)).astype(NPBF),
            }
        )
    return in_maps


def _ensure_neuron_backend():
    # if jax was already initialized cpu-only (e.g. JAX_PLATFORMS=cpu was set
    # before this module was imported), re-discover the neuron/axon backend
    import jax

    try:
        if all(d.platform == "cpu" for d in jax.devices()):
            jax.clear_backends()
    except Exception:
        pass


def kernel(q, k, v, mask, WQ_w, WQ_b, WK_w, WK_b, WV_w, WV_b, WO_w, WO_b):
    global _NC_CACHE, LAST_RESULTS
    _ensure_neuron_backend()
    if _NC_CACHE is None:
        _NC_CACHE = build_nc()
    nc = _NC_CACHE
    in_maps = prep_inputs(
        q, k, v, mask, WQ_w, WQ_b, WK_w, WK_b, WV_w, WV_b, WO_w, WO_b
    )
    try:
        res = run_bass_kernel_spmd(nc, in_maps, core_ids=list(range(B)))
    except Exception:
        # transient device wedge (NRT_EXEC_UNIT_UNRECOVERABLE seen once);
        # one retry clears it
        res = run_bass_kernel_spmd(nc, in_maps, core_ids=list(range(B)))
    LAST_RESULTS = res
    out = np.stack([res.results[b]["out"] for b in range(B)], axis=0).astype(np.float32)
    if not np.isfinite(out).all():
        # very first execution on a freshly attached core has been seen to
        # return garbage once; one retry clears it
        res = run_bass_kernel_spmd(nc, in_maps, core_ids=list(range(B)))
        LAST_RESULTS = res
        out = np.stack([res.results[b]["out"] for b in range(B)], axis=0).astype(
            np.float32
        )
    return out

